# revision 1
# baseline (speedup 1.0000x reference)
"""Trainium2 Bass kernel for the KSubspaceBaseModel objective.

Reference computes, for B=2048 samples x (B, D=1024) and subspace bases
Us (R=4, K=16, D, d=32):
    z = x @ U; x_ = z @ U^T; loss = 0.5*||x - x_||^2  (per b, r, k)
    obj_r = mean_b min_k loss

Algebraic collapse used here: with G = U^T U,
    loss = 0.5||x||^2 - z^T (I - 0.5 G) z
Folding L = chol(I - 0.5G) into U (Ut = U @ L) host-side gives
    loss = 0.5||x||^2 - ||Ut^T x||^2
so the device only computes z~ = Ut^T x, squares it, sums each subspace's
32 latent columns, and takes max_k.  obj_r = 0.5*mean||x||^2 - mean_b max_k.

Sharding over 8 cores: 4 batch quarters (512 samples) x 2 subspace halves
(32 subspaces = 2 whole replicates), so the k-max is core-local.
Device layout: stationary = x^T chunks (contraction D on partitions),
moving = Ut.  z~ lands [batch(128) x latent] in PSUM, so per-subspace sums
and the k-max are free-dim reductions.

DMA strategy: 3 coalesced 1MB input DMAs (ut half 0 + ut half 1 on the
sync HWDGE ring, xt on the scalar ring) and 2 coalesced output DMAs —
per-dma_start fixed cost is ~1-2us and transfers serialize FIFO per ring.
"""

import numpy as np
import ml_dtypes

import concourse.bass as bass
import concourse.bacc as bacc
import concourse.mybir as mybir
import concourse.tile as tile
from concourse.bass_utils import run_bass_kernel_spmd

B, D, R, K, d = 2048, 1024, 4, 16, 32
NCORES = 8
NB = B // 4          # 512 samples per core
NS = 32              # subspaces per core (2 replicates)
KC = D // 128        # 8 contraction chunks
BC = NB // 128       # 4 batch chunks per core
BF16 = mybir.dt.bfloat16
FP32 = mybir.dt.float32

_COMPILED = {}
LAST_RESULTS = None


def _build():
    nc = bacc.Bacc("TRN2", target_bir_lowering=False, debug=False)
    # inputs are pre-arranged host-side into the exact SBUF layout so each
    # partition's DMA read is one contiguous run
    xt = nc.dram_tensor("xt", [128, KC * NB], BF16, kind="ExternalInput")
    u0 = nc.dram_tensor("u0", [128, KC * 512], BF16, kind="ExternalInput")
    u1 = nc.dram_tensor("u1", [128, KC * 512], BF16, kind="ExternalInput")
    outp = nc.dram_tensor("outp", [128, KC + 2 * BC], FP32,
                          kind="ExternalOutput")

    xt_v = xt.ap().rearrange("p (o n) -> p o n", o=KC)    # [128, KC, NB]
    u_v = [u.ap().rearrange("p (o n) -> p o n", o=KC) for u in (u0, u1)]

    NPH = 4            # kc phases; 2 kc-chunks per phase
    PKC = KC // NPH

    with tile.TileContext(nc) as tc:
        with (
            tc.tile_pool(name="xsb", bufs=1) as xpool,
            tc.tile_pool(name="usb", bufs=1) as upool,
            tc.tile_pool(name="esb", bufs=3) as epool,
            tc.tile_pool(name="asb", bufs=1) as apool,
            tc.tile_pool(name="sqsb", bufs=2) as sqpool,
            tc.tile_pool(name="single", bufs=1) as spool,
            tc.tile_pool(name="zp", bufs=1, space="PSUM") as zpool,
        ):
            # Quarter-granularity input DMAs, rings balanced: sync carries xt
            # (4 x 256KB), scalar carries u0/u1 (8 x 128KB), interleaved by
            # phase so phase p's operands arrive together.
            x_q = [None] * NPH
            u_q = [[None] * NPH, [None] * NPH]
            for p in range(NPH):
                x_q[p] = xpool.tile([128, PKC, NB], BF16, tag=f"x{p}",
                                    name=f"x{p}")
                nc.sync.dma_start(x_q[p][:], xt_v[:, p * PKC:(p + 1) * PKC, :])
                for nh in range(2):
                    u_q[nh][p] = upool.tile([128, PKC, 512], BF16,
                                            tag=f"u{nh}_{p}", name=f"u{nh}_{p}")
                    nc.scalar.dma_start(u_q[nh][p][:],
                                        u_v[nh][:, p * PKC:(p + 1) * PKC, :])

            # cols 0:KC = xsq partials, cols KC: = per-(bc,nh) k-maxes
            ostage = spool.tile([128, KC + 2 * BC], FP32, tag="ostage")

            # PE warm-up: dep-free matmuls on a memset tile keep TensorE busy
            # through the DMA-wait head so HAM is un-throttled (K=8/8) when
            # the real matmuls arrive.  Shares a PSUM bank with the last-
            # emitted real group, which starts late enough to not collide.
            warm = spool.tile([128, 640], BF16, tag="warm")
            nc.vector.memset(warm[:], 0.0)
            wp = zpool.tile([128, 512], FP32, tag="zp_3_1", name="warm_ps")
            for i in range(14):
                nc.tensor.matmul(wp[:], warm[:, 0:128], warm[:, 128:640],
                                 start=True, stop=True)

            # All 8 (bc, nh) groups accumulate in their own PSUM bank across
            # the 4 kc phases; each phase only needs that quarter's inputs.
            zps = {}
            for bc in range(BC):
                for nh in range(2):
                    zps[(bc, nh)] = zpool.tile([128, 512], FP32,
                                               tag=f"zp_{bc}_{nh}",
                                               name=f"zp_{bc}_{nh}")
            for p in range(NPH):
                for i in range(PKC):
                    kc = p * PKC + i
                    for bc in range(BC):
                        lhsT = x_q[p][:, i, bc * 128:(bc + 1) * 128]
                        for nh in range(2):
                            nc.tensor.matmul(
                                zps[(bc, nh)][:], lhsT, u_q[nh][p][:, i, :],
                                start=(kc == 0), stop=(kc == KC - 1),
                                skip_group_check=True,
                            )
                    # xsq rides along on ScalarE (square + free-dim accum)
                    sq = sqpool.tile([128, NB], FP32, tag="sq")
                    nc.scalar.activation(
                        sq[:], x_q[p][:, i, :],
                        mybir.ActivationFunctionType.Square,
                        accum_out=ostage[:, kc:kc + 1],
                    )

            for bc in range(BC):
                for nh in range(2):
                    # e = z~^2 ; [128 batch, 512] -> per-subspace sums [128, 16]
                    e = epool.tile([128, 512], BF16, tag="e")
                    nc.scalar.square(e[:], zps[(bc, nh)][:])
                    a = apool.tile([128, K], FP32, tag=f"a_{bc}_{nh}",
                                   name=f"a_{bc}_{nh}")
                    nc.vector.reduce_sum(
                        a[:], e.rearrange("p (k c) -> p k c", c=d),
                        axis=mybir.AxisListType.X,
                    )
                    # col KC+2*bc+nh = max_k for sample bc*128+p, replicate nh
                    j = KC + 2 * bc + nh
                    nc.vector.reduce_max(ostage[:, j:j + 1], a[:],
                                         axis=mybir.AxisListType.X)
            nc.sync.dma_start(outp.ap()[:, :], ostage[:])

    nc.compile()
    return nc


def _prep(x, Us):
    xt_bf = np.ascontiguousarray(x.T.astype(ml_dtypes.bfloat16))       # (D, B)
    Us64 = Us.astype(np.float64)
    eye = np.eye(d)
    # fold chol(I - 0.5 U^T U) into U, all 64 subspaces at once
    G = np.einsum('skDa,skDb->skab', Us64, Us64)                        # (R,K,d,d)
    L = np.linalg.cholesky(eye[None, None] - 0.5 * G)
    Ut = np.einsum('skDa,skab->skDb', Us64, L)                          # (R,K,D,d)
    ut_all = Ut.transpose(2, 0, 1, 3).reshape(D, R * K * d)             # (D, 2048)
    ut_bf = np.ascontiguousarray(ut_all.astype(ml_dtypes.bfloat16))
    def onchip(arr):  # (D, cols) -> [128, KC*cols], partition-major
        cols = arr.shape[1]
        return np.ascontiguousarray(
            arr.reshape(KC, 128, cols).transpose(1, 0, 2).reshape(128, KC * cols))

    in_maps = []
    for c in range(NCORES):
        s2, b4 = c // 4, c % 4
        uts = ut_bf[:, 1024 * s2: 1024 * (s2 + 1)]
        in_maps.append({
            "xt": onchip(xt_bf[:, NB * b4: NB * (b4 + 1)]),
            "u0": onchip(uts[:, 0:512]),
            "u1": onchip(uts[:, 512:1024]),
        })
    return in_maps


def kernel(x, Us, _trace=False):
    global LAST_RESULTS
    if "nc" not in _COMPILED:
        _COMPILED["nc"] = _build()
    nc = _COMPILED["nc"]
    in_maps = _prep(np.asarray(x), np.asarray(Us))
    res = run_bass_kernel_spmd(nc, in_maps, core_ids=list(range(NCORES)),
                               trace=_trace)
    LAST_RESULTS = res
    S = sum(res.results[c]["outp"][:, :KC].sum(dtype=np.float64)
            for c in range(4))
    base = 0.5 * S / B
    obj = np.empty(R, np.float32)
    for r in range(R):
        s2, nh = r // 2, r % 2
        # outp[p, KC+2*bc+nh] = max_k of sample bc*128+p for replicate nh
        vals = [res.results[4 * s2 + b]["outp"][:, KC + nh::2] for b in range(4)]
        obj[r] = np.float32(base - np.mean(
            [v.astype(np.float64).mean() for v in vals]))
    return obj



# revision 3
# speedup vs baseline: 1.5010x; 1.5010x over previous
"""Trainium2 Bass kernel for the KSubspaceBaseModel objective.

Reference computes, for B=2048 samples x (B, D=1024) and subspace bases
Us (R=4, K=16, D, d=32):
    z = x @ U; x_ = z @ U^T; loss = 0.5*||x - x_||^2  (per b, r, k)
    obj_r = mean_b min_k loss

Algebraic collapse used here: with G = U^T U,
    loss = 0.5||x||^2 - z^T (I - 0.5 G) z
Folding L = chol(I - 0.5G) into U (Ut = U @ L) host-side gives
    loss = 0.5||x||^2 - ||Ut^T x||^2
so the device only computes z~ = Ut^T x, squares it, sums each subspace's
32 latent columns, and takes max_k.  obj_r = 0.5*mean||x||^2 - mean_b max_k.
The 0.5*mean||x||^2 constant is computed host-side (like the chol fold).

Sharding over 8 cores: 2 batch halves (1024 samples) x 4 replicates, so
each core owns one replicate's full 16 subspaces and the k-max is local.

Device math in fp8 e4m3 with DoubleRow perf mode (2 fp8 MACs/cell/cycle):
inputs are scaled host-side (x*8, Ut*128) to dodge fp8 subnormals; the
device output is max_k ||(128Ut)^T (8x)||^2 = 2^20 * max_k, divided out
on host.  Tolerable: obj ~ 511.5 with 2e-2 rel tolerance, and fp8 noise
on the energies is ~1e-2 absolute.

Layout: stationary = x^T chunks [128 contr x 128 samples], moving = Ut
[128 contr x 512 latent cols]; DoubleRow pairs contraction rows
(256q + 128j + p) via the middle dim of [128, 2, cols] APs.  z~ lands
[samples(128) x 512] in PSUM so the per-subspace sums and k-max are
free-dim reductions (scalar square -> vector reduce_sum/reduce_max).
Loop is group-major (bc outer, q inner) so each group's epilogue overlaps
the next group's matmuls; only the last group's epilogue is exposed.

DMA: sync ring carries ut + x pairs 01,23 (critical prefix), scalar ring
x pairs 45,67.  Warm-up matmuls keep the PE busy through the DMA-wait
head so HAM is un-throttled when the real matmuls arrive.
"""

import numpy as np
import ml_dtypes

import concourse.bass as bass
import concourse.bacc as bacc
import concourse.mybir as mybir
import concourse.tile as tile
from concourse.bass_utils import run_bass_kernel_spmd

B, D, R, K, d = 2048, 1024, 4, 16, 32
NCORES = 8
NB = B // 2          # 1024 samples per core
NQ = 4               # 256-row contraction chunks (DoubleRow pairs)
NBC = NB // 128      # 8 sample blocks per core
SX = 8.0             # x scale into fp8
SU = 128.0           # Ut scale into fp8
ESCALE = (SX * SU) ** 2
FP8 = mybir.dt.float8e4
BF16 = mybir.dt.bfloat16
FP32 = mybir.dt.float32

_COMPILED = {}
LAST_RESULTS = None


def _build():
    nc = bacc.Bacc("TRN2", target_bir_lowering=False, debug=False)
    # host-prearranged so each partition's DMA read is one contiguous run
    # xt[p, bc, q, j, s] = 8*x[1024b + 128bc + s, 256q + 128j + p]
    # ut[p, q, j, kd]    = 128*Ut[r][256q + 128j + p, kd]
    xt = nc.dram_tensor("xt", [128, NBC * NQ * 2 * 128], FP8,
                        kind="ExternalInput")
    ut = nc.dram_tensor("ut", [128, NQ * 2 * 512], FP8, kind="ExternalInput")
    outp = nc.dram_tensor("outp", [128, NBC], FP32, kind="ExternalOutput")

    xt_v = xt.ap().rearrange("p (bc q j s) -> p bc q j s", bc=NBC, q=NQ, j=2)

    with tile.TileContext(nc) as tc:
        with (
            tc.tile_pool(name="xsb", bufs=1) as xpool,
            tc.tile_pool(name="usb", bufs=1) as upool,
            tc.tile_pool(name="esb", bufs=3) as epool,
            tc.tile_pool(name="asb", bufs=2) as apool,
            tc.tile_pool(name="single", bufs=1) as spool,
            tc.tile_pool(name="zp", bufs=1, space="PSUM") as zpool,
        ):
            # Input DMAs. sync carries the critical prefix (all of ut, then
            # x pairs 0-1 and 2-3); scalar (whose first issue is delayed by
            # the activation-table load) carries the late pairs.
            u_t = upool.tile([128, NQ, 2, 512], FP8, tag="ut", name="ut")
            nc.sync.dma_start(u_t[:], ut.ap())
            x_t = [None] * (NBC // 2)
            for pr in range(NBC // 2):
                x_t[pr] = xpool.tile([128, 2, NQ, 2, 128], FP8,
                                     tag=f"x{pr}", name=f"x{pr}")
            nc.sync.dma_start(x_t[0][:], xt_v[:, 0:2])
            nc.sync.dma_start(x_t[1][:], xt_v[:, 2:4])
            nc.scalar.dma_start(x_t[2][:], xt_v[:, 4:6])
            nc.scalar.dma_start(x_t[3][:], xt_v[:, 6:8])

            ostage = spool.tile([128, NBC], FP32, tag="ostage")

            # PE warm-up: dep-free matmuls keep TensorE busy through the
            # DMA-wait head so HAM is un-throttled (K=8/8) when the real
            # matmuls arrive.  Shares the last group's PSUM bank, which
            # starts late enough to not collide.
            warm = spool.tile([128, 640], BF16, tag="warm")
            nc.gpsimd.memset(warm[:], 0.0)
            wp = zpool.tile([128, 512], FP32, tag=f"zp{NBC - 1}",
                            name="warm_ps")
            for i in range(14):
                nc.tensor.matmul(wp[:], warm[:, 0:128], warm[:, 128:640],
                                 start=True, stop=True, skip_group_check=True)

            zps = [zpool.tile([128, 512], FP32, tag=f"zp{bc}",
                              name=f"zp{bc}") for bc in range(NBC)]
            for bc in range(NBC):
                for q in range(NQ):
                    nc.tensor.matmul(
                        zps[bc][:], x_t[bc // 2][:, bc % 2, q, :, :],
                        u_t[:, q, :, :],
                        start=(q == 0), stop=(q == NQ - 1),
                        perf_mode=mybir.MatmulPerfMode.DoubleRow,
                        skip_group_check=True,
                    )
                # epilogue rides under the next group's matmuls
                e = epool.tile([128, 512], BF16, tag="e")
                nc.scalar.square(e[:], zps[bc][:])
                a = apool.tile([128, K], FP32, tag="a")
                nc.vector.reduce_sum(
                    a[:], e.rearrange("p (k c) -> p k c", c=d),
                    axis=mybir.AxisListType.X,
                )
                nc.vector.reduce_max(ostage[:, bc:bc + 1], a[:],
                                     axis=mybir.AxisListType.X)
            nc.sync.dma_start(outp.ap()[:, :], ostage[:])

    nc.compile()
    return nc


def _prep(x, Us):
    x8 = (x.astype(np.float64) * SX).astype(ml_dtypes.float8_e4m3)  # (B, D)
    Us64 = Us.astype(np.float64)
    eye = np.eye(d)
    # fold chol(I - 0.5 U^T U) into U, all 64 subspaces at once
    G = np.einsum('skDa,skDb->skab', Us64, Us64)                    # (R,K,d,d)
    L = np.linalg.cholesky(eye[None, None] - 0.5 * G)
    Ut = np.einsum('skDa,skab->skDb', Us64, L)                      # (R,K,D,d)

    in_maps = []
    for c in range(NCORES):
        r, b = c // 2, c % 2
        xq = x8[NB * b: NB * (b + 1)]                               # (NB, D)
        xa = xq.reshape(NBC, 128, NQ, 2, 128).transpose(4, 0, 2, 3, 1)
        uu = (Ut[r] * SU).transpose(1, 0, 2).reshape(D, K * d)      # (D, 512)
        ua = uu.reshape(NQ, 2, 128, K * d).transpose(2, 0, 1, 3)
        in_maps.append({
            "xt": np.ascontiguousarray(xa.reshape(128, -1)).astype(
                ml_dtypes.float8_e4m3),
            "ut": np.ascontiguousarray(ua).astype(
                ml_dtypes.float8_e4m3).reshape(128, -1),
        })
    return in_maps


def kernel(x, Us, _trace=False):
    global LAST_RESULTS
    if "nc" not in _COMPILED:
        _COMPILED["nc"] = _build()
    nc = _COMPILED["nc"]
    x = np.asarray(x)
    in_maps = _prep(x, np.asarray(Us))
    res = run_bass_kernel_spmd(nc, in_maps, core_ids=list(range(NCORES)),
                               trace=_trace)
    LAST_RESULTS = res
    base = 0.5 * np.sum(x.astype(np.float64) ** 2) / B
    obj = np.empty(R, np.float32)
    for r in range(R):
        m = np.mean([res.results[2 * r + b]["outp"].astype(np.float64).mean()
                     for b in (0, 1)])
        obj[r] = np.float32(base - m / ESCALE)
    return obj
